# revision 24
# baseline (speedup 1.0000x reference)
"""MHA decode-step kernel for Trainium2, sharded over 8 NeuronCores.

Problem: B=32, S=1, E=2048, H=16 heads of d=128, KV cache len 2048.
out = softmax((X Wq + bq) Kc^T / sqrt(d) + logmask) Vc @ Wo + bo, plus the
updated caches Kc/Vc and mask.

Sharding: tensor-parallel over heads — each core owns 2 heads (all 32
batches). Per core that means reading its 2 heads' K/V cache shard
(2 x 64 MB), the head-sliced QKV projection weights (~6 MB) and a
128-row slice of Wo. The partial output projections (each core
contributes rows h*128:(h+1)*128 of the Wo contraction) are summed on
host. The kernel is HBM-bound: ~136 MB/core at ~360 GB/s.

Device layout per core (pair p = local_head * 32 + batch, 64 pairs):
  - K cache arrives TRANSPOSED per pair: [d=128 partitions, kv] so score
    matmuls contract d on partitions; V arrives natural [kv, d] chunked
    so context matmuls contract kv on partitions.
  - scores per pair live as [128 kv-in-chunk, 16 chunks] in PSUM;
    softmax runs without max-subtraction (scores are O(5) here), the
    per-pair normalizer is assembled via a ones-vector matmul over the
    activation accumulator and applied at the very end, batched over
    all 64 pairs.
  - The appended (new-token) K/V column is handled separately: its
    score via a [1,1] matmul per pair into a shared [1,64] PSUM row,
    its V contribution via a broadcast outer-product matmul.
"""

import numpy as np
import ml_dtypes
from contextlib import ExitStack

import concourse.bass as bass
import concourse.bacc as bacc
import concourse.mybir as mybir
import concourse.tile as tile

F32 = mybir.dt.float32
BF16 = mybir.dt.bfloat16
AF = mybir.ActivationFunctionType

# Full-problem dims
B = 32          # batch
S = 1           # new tokens
E = 2048        # embed
H = 16          # heads
D = 128         # head dim
KV = 2048       # cache length
N_CORES = 8
HL = H // N_CORES   # local heads per core = 2
NEG = -1.0e30


def build_mha_body(tc, aps, dims, masked=True):
    """Emit the per-core MHA program into TileContext tc.

    aps: dict of bass.AP keyed by tensor name (see build_nc).
    dims: (b, hl, kv, e, eo)
    """
    nc = tc.nc
    b, hl, kv, e, eo = dims
    np_ = hl * b            # pairs
    nkc = kv // 128         # kv chunks
    nec = e // 128          # contraction chunks for projections

    xtb, wq, wqkv, bqkv, wo, ktc, vc = (
        aps["xtb"], aps["wq"], aps["wqkv"], aps["bqkv"],
        aps["wo"], aps["ktc"], aps["vc"])
    lmask = aps.get("lmask")
    outp, ktnew, vtnew = aps["outp"], aps["ktnew"], aps["vtnew"]

    with ExitStack() as ctx:
        singles = ctx.enter_context(tc.tile_pool(name="singles", bufs=1))
        wpool = ctx.enter_context(tc.tile_pool(name="wpool", bufs=1))
        kvpool = ctx.enter_context(tc.tile_pool(name="kvpool", bufs=18))
        epool = ctx.enter_context(tc.tile_pool(name="epool", bufs=12))
        psacc = ctx.enter_context(tc.tile_pool(name="psacc", bufs=1, space="PSUM"))

        # ---- constants / weights ----
        wq_sb = []
        for l in range(hl):
            t = wpool.tile([128, nec, D], BF16, tag=f"wq{l}")
            nc.sync.dma_start(out=t, in_=wq[l].rearrange("e (c d) -> e c d", d=D))
            wq_sb.append(t)
        xtb_sb = singles.tile([128, nec, b], BF16, tag="xtb")
        nc.sync.dma_start(out=xtb_sb, in_=xtb)
        w_sb = []
        for i in range(2 * hl):
            wt = wpool.tile([128, nec, D], BF16, tag=f"w{i}")
            w_sb.append(wt)
        wo_sb = wpool.tile([128, hl, eo], BF16, tag="wo")
        b_sb = singles.tile([D, 3 * hl], F32, tag="b")
        nc.sync.dma_start(out=b_sb, in_=bqkv[:, :])
        if masked:
            lm_sb = singles.tile([128, b, nkc], F32, tag="lm")
            nc.sync.dma_start(out=lm_sb, in_=lmask)
        ones_c = singles.tile([128, 1], F32, tag="ones_c")
        nc.vector.memset(ones_c, 1.0)
        ones_r = singles.tile([1, 128], F32, tag="ones_r")
        nc.vector.memset(ones_r, 1.0)

        # ---- QKV projection: qt/kt/vt [d=128, pairs] (pair = l*b + n) ----
        kt = singles.tile([D, np_], F32, tag="kt")
        vt = singles.tile([D, np_], F32, tag="vt")
        qtb = singles.tile([D, np_], BF16, tag="qtb")
        ktb = singles.tile([D, np_], BF16, tag="ktb")
        def proj_one(pool, t_, l, dest):
            # t_ = 1 (k) or 2 (v); bf16 weights, fp32 PSUM accumulation
            i = (t_ - 1) * hl + l
            ps = pool.tile([D, b], F32, tag="qkv")
            for c in range(nec):
                nc.tensor.matmul(
                    ps, w_sb[i][:, c, :], xtb_sb[:, c, :],
                    start=(c == 0), stop=(c == nec - 1))
            nc.vector.tensor_scalar_add(
                dest[:, l * b:(l + 1) * b], ps, b_sb[:, t_ * hl + l:t_ * hl + l + 1])

        # only the q projection gates the attention loop, and q is consumed
        # exclusively as bf16 by the score matmuls, so it runs fully in bf16;
        # k/v projections and the snew matmuls slot into the loop's idle time
        with tc.tile_pool(name="psproj", bufs=2, space="PSUM") as psproj:
            for l in range(hl):
                ps = psproj.tile([D, b], F32, tag="qkv")
                for c in range(nec):
                    nc.tensor.matmul(
                        ps, wq_sb[l][:, c, :], xtb_sb[:, c, :],
                        start=(c == 0), stop=(c == nec - 1))
                nc.vector.tensor_scalar_add(
                    qtb[:, l * b:(l + 1) * b], ps, b_sb[:, l:l + 1])

        # ---- attention over the cache, pair by pair ----
        ctxp = psacc.tile([D, np_], F32, tag="ctx")     # unnormalized context
        snew = psacc.tile([1, np_], F32, tag="snew")    # new-token scores
        accall = singles.tile([128, np_], F32, tag="accall")  # partial exp sums

        with tc.tile_pool(name="pspair", bufs=5, space="PSUM") as pspair:
            for p in range(np_):
                bi = p % b
                h2 = kv // 2
                ktile = kvpool.tile([128, kv], BF16, tag="ktile")
                nc.scalar.dma_start(out=ktile[:, 0:h2], in_=ktc[p][:, 0:h2])
                nc.sync.dma_start(out=ktile[:, h2:kv], in_=ktc[p][:, h2:kv])
                vtile = kvpool.tile([128, kv], BF16, tag="vtile")
                nc.sync.dma_start(out=vtile[:, 0:h2], in_=vc[p][:, 0:h2])
                nc.scalar.dma_start(out=vtile[:, h2:kv], in_=vc[p][:, h2:kv])

                sp = pspair.tile([128, nkc], F32, tag="sp")
                for c in range(nkc):
                    nc.tensor.matmul(
                        sp[:, c:c + 1], ktile[:, c * 128:(c + 1) * 128],
                        qtb[:, p:p + 1], start=True, stop=True)

                ep = epool.tile([128, nkc], BF16, tag="ep")
                if masked:
                    sm = epool.tile([128, nkc], F32, tag="sm")
                    nc.vector.tensor_add(sm, sp, lm_sb[:, bi, :])
                    nc.scalar.activation(
                        ep, sm, AF.Exp, accum_out=accall[:, p:p + 1])
                else:
                    nc.scalar.activation(
                        ep, sp, AF.Exp, accum_out=accall[:, p:p + 1])

                for c in range(nkc):
                    nc.tensor.matmul(
                        ctxp[:, p:p + 1], vtile[:, c * 128:(c + 1) * 128],
                        ep[:, c:c + 1], start=(c == 0), stop=(c == nkc - 1))

                # k/v projections are off the critical path: their weight
                # DMAs and PE work are spread through the loop in small bites
                if p == 2:
                    nc.sync.dma_start(
                        out=w_sb[0],
                        in_=wqkv[0].rearrange("e (c d) -> e c d", d=D))
                    nc.sync.dma_start(
                        out=w_sb[1],
                        in_=wqkv[1].rearrange("e (c d) -> e c d", d=D))
                elif p == 5:
                    nc.sync.dma_start(
                        out=w_sb[2],
                        in_=wqkv[2].rearrange("e (c d) -> e c d", d=D))
                    nc.sync.dma_start(
                        out=w_sb[3],
                        in_=wqkv[3].rearrange("e (c d) -> e c d", d=D))
                elif p == 8:
                    es_proj = ExitStack()
                    psproj2 = es_proj.enter_context(
                        tc.tile_pool(name="psproj2", bufs=1, space="PSUM"))
                    proj_one(psproj2, 1, 0, kt)
                elif p == 12:
                    proj_one(psproj2, 1, 1, kt)
                    nc.vector.tensor_copy(ktb, kt)
                elif p == 16:
                    proj_one(psproj2, 2, 0, vt)
                elif p == 20:
                    proj_one(psproj2, 2, 1, vt)
                    es_proj.close()
                elif p == 24:
                    for pp in range(np_):
                        nc.tensor.matmul(
                            snew[0:1, pp:pp + 1], ktb[:, pp:pp + 1],
                            qtb[:, pp:pp + 1], start=True, stop=True)
                elif p == 28:
                    nc.sync.dma_start(
                        out=wo_sb, in_=wo.rearrange("l d e -> d l e"))

        # ---- batched softmax tail ----
        ctxn = singles.tile([128, np_], BF16, tag="ctxn")
        with tc.tile_pool(name="pstail", bufs=1, space="PSUM") as pstail:
            sums_ps = pstail.tile([1, np_], F32, tag="sums")
            nc.tensor.matmul(sums_ps, ones_c, accall, start=True, stop=True)
            enew = singles.tile([1, np_], F32, tag="enew")
            nc.scalar.activation(enew, snew, AF.Exp)
            tot = singles.tile([1, np_], F32, tag="tot")
            nc.vector.tensor_add(tot, sums_ps, enew)
            rec = singles.tile([1, np_], F32, tag="rec")
            nc.vector.reciprocal(rec, tot)
            rbc = pstail.tile([128, np_], F32, tag="rbc")
            nc.tensor.matmul(rbc, ones_r, rec, start=True, stop=True)
            ebc = pstail.tile([128, np_], F32, tag="ebc")
            nc.tensor.matmul(ebc, ones_r, enew, start=True, stop=True)
            ctxe = singles.tile([128, np_], F32, tag="ctxe")
            nc.vector.tensor_mul(ctxe, vt, ebc)
            ctxs = singles.tile([128, np_], F32, tag="ctxs")
            nc.vector.tensor_add(ctxs, ctxp, ctxe)
            nc.vector.tensor_mul(ctxn, ctxs, rbc)   # bf16 for the out proj

        # ---- output projection (partial: this core's Wo rows) ----
        with tc.tile_pool(name="psout", bufs=1, space="PSUM") as psout:
            op_ps = psout.tile([b, eo], F32, tag="op")
            for l in range(hl):
                for n0 in range(0, eo, 512):
                    w512 = min(512, eo - n0)
                    nc.tensor.matmul(
                        op_ps[:, n0:n0 + w512], ctxn[:, l * b:(l + 1) * b],
                        wo_sb[:, l, n0:n0 + w512],
                        start=(l == 0), stop=(l == hl - 1))
            out_sb = singles.tile([b, eo], F32, tag="out")
            nc.vector.tensor_copy(out_sb, op_ps)
            nc.sync.dma_start(out=outp, in_=out_sb)
        nc.sync.dma_start(out=ktnew, in_=kt)
        nc.sync.dma_start(out=vtnew, in_=vt)


def build_nc(b=B, hl=HL, kv=KV, e=E, eo=E, masked=True):
    np_ = hl * b
    nkc = kv // 128
    nc = bacc.Bacc(None)
    t = {}
    t["xtb"] = nc.dram_tensor("xtb", [128, e // 128, b], BF16, kind="ExternalInput")
    t["wq"] = nc.dram_tensor("wq", [hl, 128, (e // 128) * D], BF16, kind="ExternalInput")
    t["wqkv"] = nc.dram_tensor("wqkv", [2 * hl, 128, (e // 128) * D], BF16, kind="ExternalInput")
    t["bqkv"] = nc.dram_tensor("bqkv", [D, 3 * hl], F32, kind="ExternalInput")
    t["wo"] = nc.dram_tensor("wo", [hl, D, eo], BF16, kind="ExternalInput")
    if masked:
        t["lmask"] = nc.dram_tensor("lmask", [128, b, nkc], F32, kind="ExternalInput")
    t["ktc"] = nc.dram_tensor("ktc", [np_, D, kv], BF16, kind="ExternalInput")
    t["vc"] = nc.dram_tensor("vc", [np_, 128, kv], BF16, kind="ExternalInput")
    t["outp"] = nc.dram_tensor("outp", [b, eo], F32, kind="ExternalOutput")
    t["ktnew"] = nc.dram_tensor("ktnew", [D, np_], F32, kind="ExternalOutput")
    t["vtnew"] = nc.dram_tensor("vtnew", [D, np_], F32, kind="ExternalOutput")
    with tile.TileContext(nc) as tc:
        build_mha_body(tc, {k: v[:] for k, v in t.items()}, (b, hl, kv, e, eo),
                       masked=masked)
    nc.finalize()
    return nc


def make_core_inputs(core, X, kcache, vcache, mask, Wqkv, bqkv):
    """Host-side shard prep for one core (numpy, full-problem dims)."""
    inv_sqrt_d = np.float32(1.0 / np.sqrt(D))
    h0 = core * HL
    heads = list(range(h0, h0 + HL))

    w_slices = []
    b_slices = []
    for t in range(3):          # q, k, v
        for h in heads:
            ws = np.ascontiguousarray(Wqkv[:, t * E + h * D: t * E + (h + 1) * D])
            bs = np.ascontiguousarray(bqkv[t * E + h * D: t * E + (h + 1) * D])
            if t == 0:
                ws = ws * inv_sqrt_d
                bs = bs * inv_sqrt_d
            w_slices.append(ws)
            b_slices.append(bs)
    # chunk-transposed so each SBUF partition row is one contiguous DMA run:
    # [E, D] -> [e_in_chunk=128, chunk, D]
    packed = [w.reshape(E // 128, 128, D).transpose(1, 0, 2).reshape(128, -1)
              for w in w_slices]
    wq_h = np.stack(packed[:HL], axis=0).astype(ml_dtypes.bfloat16)
    wqkv_h = np.stack(packed[HL:], axis=0).astype(ml_dtypes.bfloat16)
    bqkv_h = np.stack(b_slices, axis=1)                       # [D, 3*HL]

    lmask_h = np.ascontiguousarray(
        np.where(mask == 0, np.float32(NEG), np.float32(0.0)).astype(np.float32)
        .reshape(B, KV // 128, 128).transpose(2, 0, 1))     # [128, B, NKC]

    bf = ml_dtypes.bfloat16
    ktc_h = np.ascontiguousarray(
        kcache[:, heads].astype(bf).transpose(1, 0, 3, 2)).reshape(HL * B, D, KV)
    # V: [kv, d] -> [k_in_chunk=128, chunk*d] so DMA runs are contiguous and
    # SBUF cols [c*128:(c+1)*128] give the [kv=128, d=128] ctx-matmul tile
    vc_h = np.ascontiguousarray(
        vcache[:, heads].astype(bf).reshape(B, HL, KV // 128, 128, D)
        .transpose(1, 0, 3, 2, 4)).reshape(HL * B, 128, KV)
    xt_h = np.ascontiguousarray(
        X.reshape(B, E).T.reshape(E // 128, 128, B).transpose(1, 0, 2))
    return {
        "xtb": xt_h.astype(ml_dtypes.bfloat16),
        "wq": wq_h,
        "wqkv": wqkv_h,
        "bqkv": bqkv_h,
        "wo": None,   # filled by caller (needs Wo)
        "lmask": lmask_h,
        "ktc": ktc_h,
        "vc": vc_h,
    }


_NC_CACHE = {}


def _get_nc(masked):
    key = "m" if masked else "f"
    if key not in _NC_CACHE:
        _NC_CACHE[key] = build_nc(masked=masked)
    return _NC_CACHE[key]


def kernel(X, kcache, vcache, mask, Wqkv, bqkv, Wo, bo, _trace=False):
    from concourse.bass_utils import run_bass_kernel_spmd

    X = np.asarray(X, dtype=np.float32)
    kcache = np.asarray(kcache, dtype=np.float32)
    vcache = np.asarray(vcache, dtype=np.float32)
    mask = np.asarray(mask, dtype=np.float32)
    Wqkv = np.asarray(Wqkv, dtype=np.float32)
    bqkv = np.asarray(bqkv, dtype=np.float32)
    Wo = np.asarray(Wo, dtype=np.float32)
    bo = np.asarray(bo, dtype=np.float32)

    masked = not bool(np.all(mask == 1.0))
    nc = _get_nc(masked)
    in_maps = []
    for core in range(N_CORES):
        m = make_core_inputs(core, X, kcache, vcache, mask, Wqkv, bqkv)
        if not masked:
            m.pop("lmask")
        h0 = core * HL
        m["wo"] = np.ascontiguousarray(
            Wo[h0 * D:(h0 + HL) * D].reshape(HL, D, E)).astype(ml_dtypes.bfloat16)
        in_maps.append(m)

    res = run_bass_kernel_spmd(nc, in_maps, list(range(N_CORES)), trace=_trace)
    outs = res.results

    out = np.zeros((B, E), dtype=np.float32)
    K_new = np.empty((B, H, S, D), dtype=np.float32)
    V_new = np.empty((B, H, S, D), dtype=np.float32)
    for core in range(N_CORES):
        r = outs[core]
        out += r["outp"]
        for l in range(HL):
            K_new[:, core * HL + l, 0, :] = r["ktnew"][:, l * B:(l + 1) * B].T
            V_new[:, core * HL + l, 0, :] = r["vtnew"][:, l * B:(l + 1) * B].T
    out = (out + bo).reshape(B, S, E)

    Kc = np.concatenate([kcache, K_new], axis=2)
    Vc = np.concatenate([vcache, V_new], axis=2)
    mask_new = np.concatenate(
        [mask, np.ones((B, S), dtype=mask.dtype)], axis=-1)
    if _trace:
        return (out, Kc, Vc, mask_new), res
    return out, Kc, Vc, mask_new


# revision 25
# speedup vs baseline: 1.0890x; 1.0890x over previous
"""MHA decode-step kernel for Trainium2, sharded over 8 NeuronCores.

Problem: B=32, S=1, E=2048, H=16 heads of d=128, KV cache len 2048.
out = softmax((X Wq + bq) Kc^T / sqrt(d) + logmask) Vc @ Wo + bo, plus the
updated caches Kc/Vc and mask.

Sharding: tensor-parallel over heads — each core owns 2 heads (all 32
batches). Per core that means reading its 2 heads' K/V cache shard
(2 x 64 MB), the head-sliced QKV projection weights (~6 MB) and a
128-row slice of Wo. The partial output projections (each core
contributes rows h*128:(h+1)*128 of the Wo contraction) are summed on
host. The kernel is HBM-bound: ~136 MB/core at ~360 GB/s.

Device layout per core (pair p = local_head * 32 + batch, 64 pairs):
  - K cache arrives TRANSPOSED per pair: [d=128 partitions, kv] so score
    matmuls contract d on partitions; V arrives natural [kv, d] chunked
    so context matmuls contract kv on partitions.
  - scores per pair live as [128 kv-in-chunk, 16 chunks] in PSUM;
    softmax runs without max-subtraction (scores are O(5) here), the
    per-pair normalizer is assembled via a ones-vector matmul over the
    activation accumulator and applied at the very end, batched over
    all 64 pairs.
  - The appended (new-token) K/V column is handled separately: its
    score via a [1,1] matmul per pair into a shared [1,64] PSUM row,
    its V contribution via a broadcast outer-product matmul.
"""

import numpy as np
import ml_dtypes
from contextlib import ExitStack

import concourse.bass as bass
import concourse.bacc as bacc
import concourse.mybir as mybir
import concourse.tile as tile

F32 = mybir.dt.float32
BF16 = mybir.dt.bfloat16
AF = mybir.ActivationFunctionType

# Full-problem dims
B = 32          # batch
S = 1           # new tokens
E = 2048        # embed
H = 16          # heads
D = 128         # head dim
KV = 2048       # cache length
N_CORES = 8
HL = H // N_CORES   # local heads per core = 2
NEG = -1.0e30


def build_mha_body(tc, aps, dims, masked=True):
    """Emit the per-core MHA program into TileContext tc.

    aps: dict of bass.AP keyed by tensor name (see build_nc).
    dims: (b, hl, kv, e, eo)
    """
    nc = tc.nc
    b, hl, kv, e, eo = dims
    np_ = hl * b            # pairs
    nkc = kv // 128         # kv chunks
    nec = e // 128          # contraction chunks for projections

    xtb, wq, wqkv, bqkv, wo, ktc, vc = (
        aps["xtb"], aps["wq"], aps["wqkv"], aps["bqkv"],
        aps["wo"], aps["ktc"], aps["vc"])
    lmask = aps.get("lmask")
    outp, ktnew, vtnew = aps["outp"], aps["ktnew"], aps["vtnew"]

    with ExitStack() as ctx:
        singles = ctx.enter_context(tc.tile_pool(name="singles", bufs=1))
        wpool = ctx.enter_context(tc.tile_pool(name="wpool", bufs=1))
        kvpool = ctx.enter_context(tc.tile_pool(name="kvpool", bufs=18))
        epool = ctx.enter_context(tc.tile_pool(name="epool", bufs=12))
        psacc = ctx.enter_context(tc.tile_pool(name="psacc", bufs=1, space="PSUM"))

        # ---- constants / weights ----
        wq_sb = []
        for l in range(hl):
            t = wpool.tile([128, nec, D], BF16, tag=f"wq{l}")
            nc.sync.dma_start(out=t, in_=wq[l].rearrange("e (c d) -> e c d", d=D))
            wq_sb.append(t)
        xtb_sb = singles.tile([128, nec, b], BF16, tag="xtb")
        nc.sync.dma_start(out=xtb_sb, in_=xtb)
        w_sb = []
        for i in range(2 * hl):
            wt = wpool.tile([128, nec, D], BF16, tag=f"w{i}")
            w_sb.append(wt)
        wo_sb = wpool.tile([128, hl, eo], BF16, tag="wo")
        b_sb = singles.tile([D, 3 * hl], F32, tag="b")
        nc.sync.dma_start(out=b_sb, in_=bqkv[:, :])
        if masked:
            lm_sb = singles.tile([128, b, nkc], F32, tag="lm")
            nc.sync.dma_start(out=lm_sb, in_=lmask)
        ones_c = singles.tile([128, 1], F32, tag="ones_c")
        nc.vector.memset(ones_c, 1.0)
        ones_r = singles.tile([1, 128], F32, tag="ones_r")
        nc.vector.memset(ones_r, 1.0)

        # ---- QKV projection: qt/kt/vt [d=128, pairs] (pair = l*b + n) ----
        kt = singles.tile([D, np_], F32, tag="kt")
        vt = singles.tile([D, np_], F32, tag="vt")
        qtb = singles.tile([D, np_], BF16, tag="qtb")
        ktb = singles.tile([D, np_], BF16, tag="ktb")
        def proj_one(pool, t_, l, dest):
            # t_ = 1 (k) or 2 (v); bf16 weights, fp32 PSUM accumulation
            i = (t_ - 1) * hl + l
            ps = pool.tile([D, b], F32, tag="qkv")
            for c in range(nec):
                nc.tensor.matmul(
                    ps, w_sb[i][:, c, :], xtb_sb[:, c, :],
                    start=(c == 0), stop=(c == nec - 1))
            nc.vector.tensor_scalar_add(
                dest[:, l * b:(l + 1) * b], ps, b_sb[:, t_ * hl + l:t_ * hl + l + 1])

        # only the q projection gates the attention loop, and q is consumed
        # exclusively as bf16 by the score matmuls, so it runs fully in bf16;
        # k/v projections and the snew matmuls slot into the loop's idle time
        with tc.tile_pool(name="psproj", bufs=2, space="PSUM") as psproj:
            for l in range(hl):
                ps = psproj.tile([D, b], F32, tag="qkv")
                for c in range(nec):
                    nc.tensor.matmul(
                        ps, wq_sb[l][:, c, :], xtb_sb[:, c, :],
                        start=(c == 0), stop=(c == nec - 1))
                nc.vector.tensor_scalar_add(
                    qtb[:, l * b:(l + 1) * b], ps, b_sb[:, l:l + 1])

        # ---- attention over the cache, pair by pair ----
        ctxp = psacc.tile([D, np_], F32, tag="ctx")     # unnormalized context
        snew = psacc.tile([1, np_], F32, tag="snew")    # new-token scores
        accall = singles.tile([128, np_], F32, tag="accall")  # partial exp sums

        with tc.tile_pool(name="pspair", bufs=5, space="PSUM") as pspair:
            for p in range(np_):
                bi = p % b
                h2 = kv // 2
                ktile = kvpool.tile([128, kv], BF16, tag="ktile")
                nc.scalar.dma_start(out=ktile[:, 0:h2], in_=ktc[p][:, 0:h2])
                nc.sync.dma_start(out=ktile[:, h2:kv], in_=ktc[p][:, h2:kv])
                vtile = kvpool.tile([128, kv], BF16, tag="vtile")
                nc.sync.dma_start(out=vtile[:, 0:h2], in_=vc[p][:, 0:h2])
                nc.scalar.dma_start(out=vtile[:, h2:kv], in_=vc[p][:, h2:kv])

                sp = pspair.tile([128, nkc], F32, tag="sp")
                for c in range(nkc):
                    nc.tensor.matmul(
                        sp[:, c:c + 1], ktile[:, c * 128:(c + 1) * 128],
                        qtb[:, p:p + 1], start=True, stop=True)

                ep = epool.tile([128, nkc], BF16, tag="ep")
                if masked:
                    sm = epool.tile([128, nkc], F32, tag="sm")
                    nc.vector.tensor_add(sm, sp, lm_sb[:, bi, :])
                    nc.scalar.activation(
                        ep, sm, AF.Exp, accum_out=accall[:, p:p + 1])
                else:
                    nc.scalar.activation(
                        ep, sp, AF.Exp, accum_out=accall[:, p:p + 1])

                for c in range(nkc):
                    nc.tensor.matmul(
                        ctxp[:, p:p + 1], vtile[:, c * 128:(c + 1) * 128],
                        ep[:, c:c + 1], start=(c == 0), stop=(c == nkc - 1))

                # k/v projections are off the critical path: their weight
                # DMAs and PE work are spread through the loop in small bites
                if p == 2:
                    nc.sync.dma_start(
                        out=w_sb[0],
                        in_=wqkv[0].rearrange("e (c d) -> e c d", d=D))
                    nc.sync.dma_start(
                        out=w_sb[1],
                        in_=wqkv[1].rearrange("e (c d) -> e c d", d=D))
                elif p == 5:
                    nc.sync.dma_start(
                        out=w_sb[2],
                        in_=wqkv[2].rearrange("e (c d) -> e c d", d=D))
                    nc.sync.dma_start(
                        out=w_sb[3],
                        in_=wqkv[3].rearrange("e (c d) -> e c d", d=D))
                elif p == 8:
                    es_proj = ExitStack()
                    psproj2 = es_proj.enter_context(
                        tc.tile_pool(name="psproj2", bufs=1, space="PSUM"))
                    proj_one(psproj2, 1, 0, kt)
                elif p == 12:
                    proj_one(psproj2, 1, 1, kt)
                    nc.vector.tensor_copy(ktb, kt)
                elif p == 16:
                    proj_one(psproj2, 2, 0, vt)
                elif p == 20:
                    proj_one(psproj2, 2, 1, vt)
                    es_proj.close()
                elif p == 24:
                    for pp in range(np_):
                        nc.tensor.matmul(
                            snew[0:1, pp:pp + 1], ktb[:, pp:pp + 1],
                            qtb[:, pp:pp + 1], start=True, stop=True)
                elif p == 28:
                    nc.sync.dma_start(
                        out=wo_sb, in_=wo.rearrange("l d e -> d l e"))

        # ---- batched softmax tail ----
        ctxn = singles.tile([128, np_], BF16, tag="ctxn")
        with tc.tile_pool(name="pstail", bufs=1, space="PSUM") as pstail:
            sums_ps = pstail.tile([1, np_], F32, tag="sums")
            nc.tensor.matmul(sums_ps, ones_c, accall, start=True, stop=True)
            enew = singles.tile([1, np_], F32, tag="enew")
            nc.scalar.activation(enew, snew, AF.Exp)
            tot = singles.tile([1, np_], F32, tag="tot")
            nc.vector.tensor_add(tot, sums_ps, enew)
            rec = singles.tile([1, np_], F32, tag="rec")
            nc.vector.reciprocal(rec, tot)
            rbc = pstail.tile([128, np_], F32, tag="rbc")
            nc.tensor.matmul(rbc, ones_r, rec, start=True, stop=True)
            ebc = pstail.tile([128, np_], F32, tag="ebc")
            nc.tensor.matmul(ebc, ones_r, enew, start=True, stop=True)
            ctxe = singles.tile([128, np_], F32, tag="ctxe")
            nc.vector.tensor_mul(ctxe, vt, ebc)
            ctxs = singles.tile([128, np_], F32, tag="ctxs")
            nc.vector.tensor_add(ctxs, ctxp, ctxe)
            nc.vector.tensor_mul(ctxn, ctxs, rbc)   # bf16 for the out proj

        # ---- output projection (partial: this core's Wo rows) ----
        with tc.tile_pool(name="psout", bufs=1, space="PSUM") as psout:
            op_ps = psout.tile([b, eo], F32, tag="op")
            for l in range(hl):
                for n0 in range(0, eo, 512):
                    w512 = min(512, eo - n0)
                    nc.tensor.matmul(
                        op_ps[:, n0:n0 + w512], ctxn[:, l * b:(l + 1) * b],
                        wo_sb[:, l, n0:n0 + w512],
                        start=(l == 0), stop=(l == hl - 1))
            out_sb = singles.tile([b, eo], F32, tag="out")
            nc.vector.tensor_copy(out_sb, op_ps)
            nc.sync.dma_start(out=outp, in_=out_sb)
        nc.sync.dma_start(out=ktnew, in_=kt)
        nc.sync.dma_start(out=vtnew, in_=vt)


def build_nc(b=B, hl=HL, kv=KV, e=E, eo=E, masked=True):
    np_ = hl * b
    nkc = kv // 128
    nc = bacc.Bacc(None)
    t = {}
    t["xtb"] = nc.dram_tensor("xtb", [128, e // 128, b], BF16, kind="ExternalInput")
    t["wq"] = nc.dram_tensor("wq", [hl, 128, (e // 128) * D], BF16, kind="ExternalInput")
    t["wqkv"] = nc.dram_tensor("wqkv", [2 * hl, 128, (e // 128) * D], BF16, kind="ExternalInput")
    t["bqkv"] = nc.dram_tensor("bqkv", [D, 3 * hl], F32, kind="ExternalInput")
    t["wo"] = nc.dram_tensor("wo", [hl, D, eo], BF16, kind="ExternalInput")
    if masked:
        t["lmask"] = nc.dram_tensor("lmask", [128, b, nkc], F32, kind="ExternalInput")
    t["ktc"] = nc.dram_tensor("ktc", [np_, D, kv], BF16, kind="ExternalInput")
    t["vc"] = nc.dram_tensor("vc", [np_, 128, kv], BF16, kind="ExternalInput")
    t["outp"] = nc.dram_tensor("outp", [b, eo], F32, kind="ExternalOutput")
    t["ktnew"] = nc.dram_tensor("ktnew", [D, np_], F32, kind="ExternalOutput")
    t["vtnew"] = nc.dram_tensor("vtnew", [D, np_], F32, kind="ExternalOutput")
    with tile.TileContext(nc) as tc:
        build_mha_body(tc, {k: v[:] for k, v in t.items()}, (b, hl, kv, e, eo),
                       masked=masked)
    nc.finalize()
    return nc


def make_core_inputs(core, X, kcache, vcache, mask, Wqkv, bqkv):
    """Host-side shard prep for one core (numpy, full-problem dims)."""
    inv_sqrt_d = np.float32(1.0 / np.sqrt(D))
    h0 = core * HL
    heads = list(range(h0, h0 + HL))

    w_slices = []
    b_slices = []
    for t in range(3):          # q, k, v
        for h in heads:
            ws = np.ascontiguousarray(Wqkv[:, t * E + h * D: t * E + (h + 1) * D])
            bs = np.ascontiguousarray(bqkv[t * E + h * D: t * E + (h + 1) * D])
            if t == 0:
                ws = ws * inv_sqrt_d
                bs = bs * inv_sqrt_d
            w_slices.append(ws)
            b_slices.append(bs)
    # chunk-transposed so each SBUF partition row is one contiguous DMA run:
    # [E, D] -> [e_in_chunk=128, chunk, D]
    packed = [w.reshape(E // 128, 128, D).transpose(1, 0, 2).reshape(128, -1)
              for w in w_slices]
    wq_h = np.stack(packed[:HL], axis=0).astype(ml_dtypes.bfloat16)
    wqkv_h = np.stack(packed[HL:], axis=0).astype(ml_dtypes.bfloat16)
    bqkv_h = np.stack(b_slices, axis=1)                       # [D, 3*HL]

    lmask_h = np.ascontiguousarray(
        np.where(mask == 0, np.float32(NEG), np.float32(0.0)).astype(np.float32)
        .reshape(B, KV // 128, 128).transpose(2, 0, 1))     # [128, B, NKC]

    bf = ml_dtypes.bfloat16
    ktc_h = np.ascontiguousarray(
        kcache[:, heads].astype(bf).transpose(1, 0, 3, 2)).reshape(HL * B, D, KV)
    # V: [kv, d] -> [k_in_chunk=128, chunk*d] so DMA runs are contiguous and
    # SBUF cols [c*128:(c+1)*128] give the [kv=128, d=128] ctx-matmul tile
    vc_h = np.ascontiguousarray(
        vcache[:, heads].astype(bf).reshape(B, HL, KV // 128, 128, D)
        .transpose(1, 0, 3, 2, 4)).reshape(HL * B, 128, KV)
    xt_h = np.ascontiguousarray(
        X.reshape(B, E).T.reshape(E // 128, 128, B).transpose(1, 0, 2))
    return {
        "xtb": xt_h.astype(ml_dtypes.bfloat16),
        "wq": wq_h,
        "wqkv": wqkv_h,
        "bqkv": bqkv_h,
        "wo": None,   # filled by caller (needs Wo)
        "lmask": lmask_h,
        "ktc": ktc_h,
        "vc": vc_h,
    }


_NC_CACHE = {}


def _get_nc(masked):
    key = "m" if masked else "f"
    if key not in _NC_CACHE:
        _NC_CACHE[key] = build_nc(masked=masked)
    return _NC_CACHE[key]


def kernel(X, kcache, vcache, mask, Wqkv, bqkv, Wo, bo, _trace=False):
    from concourse.bass_utils import run_bass_kernel_spmd

    X = np.asarray(X, dtype=np.float32)
    kcache = np.asarray(kcache, dtype=np.float32)
    vcache = np.asarray(vcache, dtype=np.float32)
    mask = np.asarray(mask, dtype=np.float32)
    Wqkv = np.asarray(Wqkv, dtype=np.float32)
    bqkv = np.asarray(bqkv, dtype=np.float32)
    Wo = np.asarray(Wo, dtype=np.float32)
    bo = np.asarray(bo, dtype=np.float32)

    masked = True   # PSUM-direct exp (masked=False) measured slower
    nc = _get_nc(masked)
    in_maps = []
    for core in range(N_CORES):
        m = make_core_inputs(core, X, kcache, vcache, mask, Wqkv, bqkv)
        if not masked:
            m.pop("lmask")
        h0 = core * HL
        m["wo"] = np.ascontiguousarray(
            Wo[h0 * D:(h0 + HL) * D].reshape(HL, D, E)).astype(ml_dtypes.bfloat16)
        in_maps.append(m)

    res = run_bass_kernel_spmd(nc, in_maps, list(range(N_CORES)), trace=_trace)
    outs = res.results

    out = np.zeros((B, E), dtype=np.float32)
    K_new = np.empty((B, H, S, D), dtype=np.float32)
    V_new = np.empty((B, H, S, D), dtype=np.float32)
    for core in range(N_CORES):
        r = outs[core]
        out += r["outp"]
        for l in range(HL):
            K_new[:, core * HL + l, 0, :] = r["ktnew"][:, l * B:(l + 1) * B].T
            V_new[:, core * HL + l, 0, :] = r["vtnew"][:, l * B:(l + 1) * B].T
    out = (out + bo).reshape(B, S, E)

    Kc = np.concatenate([kcache, K_new], axis=2)
    Vc = np.concatenate([vcache, V_new], axis=2)
    mask_new = np.concatenate(
        [mask, np.ones((B, S), dtype=mask.dtype)], axis=-1)
    if _trace:
        return (out, Kc, Vc, mask_new), res
    return out, Kc, Vc, mask_new


# revision 26
# speedup vs baseline: 1.2090x; 1.1103x over previous
"""MHA decode-step kernel for Trainium2, sharded over 8 NeuronCores.

Problem: B=32, S=1, E=2048, H=16 heads of d=128, KV cache len 2048.
out = softmax((X Wq + bq) Kc^T / sqrt(d) + logmask) Vc @ Wo + bo, plus the
updated caches Kc/Vc and mask.

Sharding: tensor-parallel over heads — each core owns 2 heads (all 32
batches). Per core that means reading its 2 heads' K/V cache shard
(2 x 64 MB), the head-sliced QKV projection weights (~6 MB) and a
128-row slice of Wo. The partial output projections (each core
contributes rows h*128:(h+1)*128 of the Wo contraction) are summed on
host. The kernel is HBM-bound: ~136 MB/core at ~360 GB/s.

Device layout per core (pair p = local_head * 32 + batch, 64 pairs):
  - K cache arrives TRANSPOSED per pair: [d=128 partitions, kv] so score
    matmuls contract d on partitions; V arrives natural [kv, d] chunked
    so context matmuls contract kv on partitions.
  - scores per pair live as [128 kv-in-chunk, 16 chunks] in PSUM;
    softmax runs without max-subtraction (scores are O(5) here), the
    per-pair normalizer is assembled via a ones-vector matmul over the
    activation accumulator and applied at the very end, batched over
    all 64 pairs.
  - The appended (new-token) K/V column is handled separately: its
    score via a [1,1] matmul per pair into a shared [1,64] PSUM row,
    its V contribution via a broadcast outer-product matmul.
"""

import numpy as np
import ml_dtypes
from contextlib import ExitStack

import concourse.bass as bass
import concourse.bacc as bacc
import concourse.mybir as mybir
import concourse.tile as tile

F32 = mybir.dt.float32
BF16 = mybir.dt.bfloat16
AF = mybir.ActivationFunctionType

# Full-problem dims
B = 32          # batch
S = 1           # new tokens
E = 2048        # embed
H = 16          # heads
D = 128         # head dim
KV = 2048       # cache length
N_CORES = 8
HL = H // N_CORES   # local heads per core = 2
NEG = -1.0e30


def build_mha_body(tc, aps, dims, masked=True):
    """Emit the per-core MHA program into TileContext tc.

    aps: dict of bass.AP keyed by tensor name (see build_nc).
    dims: (b, hl, kv, e, eo)
    """
    nc = tc.nc
    b, hl, kv, e, eo = dims
    np_ = hl * b            # pairs
    nkc = kv // 128         # kv chunks
    nec = e // 128          # contraction chunks for projections

    xtb, wq, wqkv, bqkv, wo, ktc, vc = (
        aps["xtb"], aps["wq"], aps["wqkv"], aps["bqkv"],
        aps["wo"], aps["ktc"], aps["vc"])
    lmask = aps.get("lmask")
    outp, ktnew, vtnew = aps["outp"], aps["ktnew"], aps["vtnew"]

    with ExitStack() as ctx:
        singles = ctx.enter_context(tc.tile_pool(name="singles", bufs=1))
        wpool = ctx.enter_context(tc.tile_pool(name="wpool", bufs=1))
        kvpool = ctx.enter_context(tc.tile_pool(name="kvpool", bufs=18))
        epool = ctx.enter_context(tc.tile_pool(name="epool", bufs=12))
        psacc = ctx.enter_context(tc.tile_pool(name="psacc", bufs=1, space="PSUM"))

        # ---- constants / weights ----
        wq_sb = []
        for l in range(hl):
            t = wpool.tile([128, nec, D], BF16, tag=f"wq{l}")
            nc.sync.dma_start(out=t, in_=wq[l].rearrange("e (c d) -> e c d", d=D))
            wq_sb.append(t)
        xtb_sb = singles.tile([128, nec, b], BF16, tag="xtb")
        nc.sync.dma_start(out=xtb_sb, in_=xtb)
        w_sb = []
        for i in range(2 * hl):
            wt = wpool.tile([128, nec, D], BF16, tag=f"w{i}")
            w_sb.append(wt)
        wo_sb = wpool.tile([128, hl, eo], BF16, tag="wo")
        b_sb = singles.tile([D, 3 * hl], F32, tag="b")
        nc.sync.dma_start(out=b_sb, in_=bqkv[:, :])
        if masked:
            lm_sb = singles.tile([128, b, nkc], F32, tag="lm")
            nc.sync.dma_start(out=lm_sb, in_=lmask)
        ones_c = singles.tile([128, 1], F32, tag="ones_c")
        nc.vector.memset(ones_c, 1.0)
        ones_r = singles.tile([1, 128], F32, tag="ones_r")
        nc.vector.memset(ones_r, 1.0)

        # ---- QKV projection: qt/kt/vt [d=128, pairs] (pair = l*b + n) ----
        kt = singles.tile([D, np_], F32, tag="kt")
        vt = singles.tile([D, np_], F32, tag="vt")
        qtb = singles.tile([D, np_], BF16, tag="qtb")
        ktb = singles.tile([D, np_], BF16, tag="ktb")
        def proj_one(pool, t_, l, dest):
            # t_ = 1 (k) or 2 (v); bf16 weights, fp32 PSUM accumulation
            i = (t_ - 1) * hl + l
            ps = pool.tile([D, b], F32, tag="qkv")
            for c in range(nec):
                nc.tensor.matmul(
                    ps, w_sb[i][:, c, :], xtb_sb[:, c, :],
                    start=(c == 0), stop=(c == nec - 1))
            nc.vector.tensor_scalar_add(
                dest[:, l * b:(l + 1) * b], ps, b_sb[:, t_ * hl + l:t_ * hl + l + 1])

        # only the q projection gates the attention loop, and q is consumed
        # exclusively as bf16 by the score matmuls, so it runs fully in bf16;
        # k/v projections and the snew matmuls slot into the loop's idle time
        with tc.tile_pool(name="psproj", bufs=2, space="PSUM") as psproj:
            for l in range(hl):
                ps = psproj.tile([D, b], F32, tag="qkv")
                for c in range(nec):
                    nc.tensor.matmul(
                        ps, wq_sb[l][:, c, :], xtb_sb[:, c, :],
                        start=(c == 0), stop=(c == nec - 1))
                nc.vector.tensor_scalar_add(
                    qtb[:, l * b:(l + 1) * b], ps, b_sb[:, l:l + 1])

        # ---- attention over the cache, pair by pair ----
        ctxp = psacc.tile([D, np_], F32, tag="ctx")     # unnormalized context
        snew = psacc.tile([1, np_], F32, tag="snew")    # new-token scores
        accall = singles.tile([128, np_], F32, tag="accall")  # partial exp sums

        with tc.tile_pool(name="pspair", bufs=5, space="PSUM") as pspair:
            for p in range(np_):
                bi = p % b
                ktile = kvpool.tile([128, kv], BF16, tag="ktile")
                nc.sync.dma_start(out=ktile, in_=ktc[p])
                vtile = kvpool.tile([128, kv], BF16, tag="vtile")
                nc.sync.dma_start(out=vtile, in_=vc[p])

                sp = pspair.tile([128, nkc], F32, tag="sp")
                for c in range(nkc):
                    nc.tensor.matmul(
                        sp[:, c:c + 1], ktile[:, c * 128:(c + 1) * 128],
                        qtb[:, p:p + 1], start=True, stop=True)

                ep = epool.tile([128, nkc], BF16, tag="ep")
                if masked:
                    sm = epool.tile([128, nkc], F32, tag="sm")
                    nc.vector.tensor_add(sm, sp, lm_sb[:, bi, :])
                    nc.scalar.activation(
                        ep, sm, AF.Exp, accum_out=accall[:, p:p + 1])
                else:
                    nc.scalar.activation(
                        ep, sp, AF.Exp, accum_out=accall[:, p:p + 1])

                for c in range(nkc):
                    nc.tensor.matmul(
                        ctxp[:, p:p + 1], vtile[:, c * 128:(c + 1) * 128],
                        ep[:, c:c + 1], start=(c == 0), stop=(c == nkc - 1))

                # k/v projections are off the critical path: their weight
                # DMAs and PE work are spread through the loop in small bites
                if p == 2:
                    nc.sync.dma_start(
                        out=w_sb[0],
                        in_=wqkv[0].rearrange("e (c d) -> e c d", d=D))
                    nc.sync.dma_start(
                        out=w_sb[1],
                        in_=wqkv[1].rearrange("e (c d) -> e c d", d=D))
                elif p == 5:
                    nc.sync.dma_start(
                        out=w_sb[2],
                        in_=wqkv[2].rearrange("e (c d) -> e c d", d=D))
                    nc.sync.dma_start(
                        out=w_sb[3],
                        in_=wqkv[3].rearrange("e (c d) -> e c d", d=D))
                elif p == 8:
                    es_proj = ExitStack()
                    psproj2 = es_proj.enter_context(
                        tc.tile_pool(name="psproj2", bufs=1, space="PSUM"))
                    proj_one(psproj2, 1, 0, kt)
                elif p == 12:
                    proj_one(psproj2, 1, 1, kt)
                    nc.vector.tensor_copy(ktb, kt)
                elif p == 16:
                    proj_one(psproj2, 2, 0, vt)
                elif p == 20:
                    proj_one(psproj2, 2, 1, vt)
                    es_proj.close()
                elif p == 24:
                    for pp in range(np_):
                        nc.tensor.matmul(
                            snew[0:1, pp:pp + 1], ktb[:, pp:pp + 1],
                            qtb[:, pp:pp + 1], start=True, stop=True)
                elif p == 28:
                    nc.sync.dma_start(
                        out=wo_sb, in_=wo.rearrange("l d e -> d l e"))

        # ---- batched softmax tail ----
        ctxn = singles.tile([128, np_], BF16, tag="ctxn")
        with tc.tile_pool(name="pstail", bufs=1, space="PSUM") as pstail:
            sums_ps = pstail.tile([1, np_], F32, tag="sums")
            nc.tensor.matmul(sums_ps, ones_c, accall, start=True, stop=True)
            enew = singles.tile([1, np_], F32, tag="enew")
            nc.scalar.activation(enew, snew, AF.Exp)
            tot = singles.tile([1, np_], F32, tag="tot")
            nc.vector.tensor_add(tot, sums_ps, enew)
            rec = singles.tile([1, np_], F32, tag="rec")
            nc.vector.reciprocal(rec, tot)
            rbc = pstail.tile([128, np_], F32, tag="rbc")
            nc.tensor.matmul(rbc, ones_r, rec, start=True, stop=True)
            ebc = pstail.tile([128, np_], F32, tag="ebc")
            nc.tensor.matmul(ebc, ones_r, enew, start=True, stop=True)
            ctxe = singles.tile([128, np_], F32, tag="ctxe")
            nc.vector.tensor_mul(ctxe, vt, ebc)
            ctxs = singles.tile([128, np_], F32, tag="ctxs")
            nc.vector.tensor_add(ctxs, ctxp, ctxe)
            nc.vector.tensor_mul(ctxn, ctxs, rbc)   # bf16 for the out proj

        # ---- output projection (partial: this core's Wo rows) ----
        with tc.tile_pool(name="psout", bufs=1, space="PSUM") as psout:
            op_ps = psout.tile([b, eo], F32, tag="op")
            for l in range(hl):
                for n0 in range(0, eo, 512):
                    w512 = min(512, eo - n0)
                    nc.tensor.matmul(
                        op_ps[:, n0:n0 + w512], ctxn[:, l * b:(l + 1) * b],
                        wo_sb[:, l, n0:n0 + w512],
                        start=(l == 0), stop=(l == hl - 1))
            out_sb = singles.tile([b, eo], F32, tag="out")
            nc.vector.tensor_copy(out_sb, op_ps)
            nc.sync.dma_start(out=outp, in_=out_sb)
        nc.sync.dma_start(out=ktnew, in_=kt)
        nc.sync.dma_start(out=vtnew, in_=vt)


def build_nc(b=B, hl=HL, kv=KV, e=E, eo=E, masked=True):
    np_ = hl * b
    nkc = kv // 128
    nc = bacc.Bacc(None)
    t = {}
    t["xtb"] = nc.dram_tensor("xtb", [128, e // 128, b], BF16, kind="ExternalInput")
    t["wq"] = nc.dram_tensor("wq", [hl, 128, (e // 128) * D], BF16, kind="ExternalInput")
    t["wqkv"] = nc.dram_tensor("wqkv", [2 * hl, 128, (e // 128) * D], BF16, kind="ExternalInput")
    t["bqkv"] = nc.dram_tensor("bqkv", [D, 3 * hl], F32, kind="ExternalInput")
    t["wo"] = nc.dram_tensor("wo", [hl, D, eo], BF16, kind="ExternalInput")
    if masked:
        t["lmask"] = nc.dram_tensor("lmask", [128, b, nkc], F32, kind="ExternalInput")
    t["ktc"] = nc.dram_tensor("ktc", [np_, D, kv], BF16, kind="ExternalInput")
    t["vc"] = nc.dram_tensor("vc", [np_, 128, kv], BF16, kind="ExternalInput")
    t["outp"] = nc.dram_tensor("outp", [b, eo], F32, kind="ExternalOutput")
    t["ktnew"] = nc.dram_tensor("ktnew", [D, np_], F32, kind="ExternalOutput")
    t["vtnew"] = nc.dram_tensor("vtnew", [D, np_], F32, kind="ExternalOutput")
    with tile.TileContext(nc) as tc:
        build_mha_body(tc, {k: v[:] for k, v in t.items()}, (b, hl, kv, e, eo),
                       masked=masked)
    nc.finalize()
    return nc


def make_core_inputs(core, X, kcache, vcache, mask, Wqkv, bqkv):
    """Host-side shard prep for one core (numpy, full-problem dims)."""
    inv_sqrt_d = np.float32(1.0 / np.sqrt(D))
    h0 = core * HL
    heads = list(range(h0, h0 + HL))

    w_slices = []
    b_slices = []
    for t in range(3):          # q, k, v
        for h in heads:
            ws = np.ascontiguousarray(Wqkv[:, t * E + h * D: t * E + (h + 1) * D])
            bs = np.ascontiguousarray(bqkv[t * E + h * D: t * E + (h + 1) * D])
            if t == 0:
                ws = ws * inv_sqrt_d
                bs = bs * inv_sqrt_d
            w_slices.append(ws)
            b_slices.append(bs)
    # chunk-transposed so each SBUF partition row is one contiguous DMA run:
    # [E, D] -> [e_in_chunk=128, chunk, D]
    packed = [w.reshape(E // 128, 128, D).transpose(1, 0, 2).reshape(128, -1)
              for w in w_slices]
    wq_h = np.stack(packed[:HL], axis=0).astype(ml_dtypes.bfloat16)
    wqkv_h = np.stack(packed[HL:], axis=0).astype(ml_dtypes.bfloat16)
    bqkv_h = np.stack(b_slices, axis=1)                       # [D, 3*HL]

    lmask_h = np.ascontiguousarray(
        np.where(mask == 0, np.float32(NEG), np.float32(0.0)).astype(np.float32)
        .reshape(B, KV // 128, 128).transpose(2, 0, 1))     # [128, B, NKC]

    bf = ml_dtypes.bfloat16
    ktc_h = np.ascontiguousarray(
        kcache[:, heads].astype(bf).transpose(1, 0, 3, 2)).reshape(HL * B, D, KV)
    # V: [kv, d] -> [k_in_chunk=128, chunk*d] so DMA runs are contiguous and
    # SBUF cols [c*128:(c+1)*128] give the [kv=128, d=128] ctx-matmul tile
    vc_h = np.ascontiguousarray(
        vcache[:, heads].astype(bf).reshape(B, HL, KV // 128, 128, D)
        .transpose(1, 0, 3, 2, 4)).reshape(HL * B, 128, KV)
    xt_h = np.ascontiguousarray(
        X.reshape(B, E).T.reshape(E // 128, 128, B).transpose(1, 0, 2))
    return {
        "xtb": xt_h.astype(ml_dtypes.bfloat16),
        "wq": wq_h,
        "wqkv": wqkv_h,
        "bqkv": bqkv_h,
        "wo": None,   # filled by caller (needs Wo)
        "lmask": lmask_h,
        "ktc": ktc_h,
        "vc": vc_h,
    }


_NC_CACHE = {}


def _get_nc(masked):
    key = "m" if masked else "f"
    if key not in _NC_CACHE:
        _NC_CACHE[key] = build_nc(masked=masked)
    return _NC_CACHE[key]


def kernel(X, kcache, vcache, mask, Wqkv, bqkv, Wo, bo, _trace=False):
    from concourse.bass_utils import run_bass_kernel_spmd

    X = np.asarray(X, dtype=np.float32)
    kcache = np.asarray(kcache, dtype=np.float32)
    vcache = np.asarray(vcache, dtype=np.float32)
    mask = np.asarray(mask, dtype=np.float32)
    Wqkv = np.asarray(Wqkv, dtype=np.float32)
    bqkv = np.asarray(bqkv, dtype=np.float32)
    Wo = np.asarray(Wo, dtype=np.float32)
    bo = np.asarray(bo, dtype=np.float32)

    masked = True   # PSUM-direct exp (masked=False) measured slower
    nc = _get_nc(masked)
    in_maps = []
    for core in range(N_CORES):
        m = make_core_inputs(core, X, kcache, vcache, mask, Wqkv, bqkv)
        if not masked:
            m.pop("lmask")
        h0 = core * HL
        m["wo"] = np.ascontiguousarray(
            Wo[h0 * D:(h0 + HL) * D].reshape(HL, D, E)).astype(ml_dtypes.bfloat16)
        in_maps.append(m)

    res = run_bass_kernel_spmd(nc, in_maps, list(range(N_CORES)), trace=_trace)
    outs = res.results

    out = np.zeros((B, E), dtype=np.float32)
    K_new = np.empty((B, H, S, D), dtype=np.float32)
    V_new = np.empty((B, H, S, D), dtype=np.float32)
    for core in range(N_CORES):
        r = outs[core]
        out += r["outp"]
        for l in range(HL):
            K_new[:, core * HL + l, 0, :] = r["ktnew"][:, l * B:(l + 1) * B].T
            V_new[:, core * HL + l, 0, :] = r["vtnew"][:, l * B:(l + 1) * B].T
    out = (out + bo).reshape(B, S, E)

    Kc = np.concatenate([kcache, K_new], axis=2)
    Vc = np.concatenate([vcache, V_new], axis=2)
    mask_new = np.concatenate(
        [mask, np.ones((B, S), dtype=mask.dtype)], axis=-1)
    if _trace:
        return (out, Kc, Vc, mask_new), res
    return out, Kc, Vc, mask_new


# revision 27
# speedup vs baseline: 1.2150x; 1.0049x over previous
"""MHA decode-step kernel for Trainium2, sharded over 8 NeuronCores.

Problem: B=32, S=1, E=2048, H=16 heads of d=128, KV cache len 2048.
out = softmax((X Wq + bq) Kc^T / sqrt(d) + logmask) Vc @ Wo + bo, plus the
updated caches Kc/Vc and mask.

Sharding: tensor-parallel over heads — each core owns 2 heads (all 32
batches). Per core that means reading its 2 heads' K/V cache shard
(2 x 64 MB), the head-sliced QKV projection weights (~6 MB) and a
128-row slice of Wo. The partial output projections (each core
contributes rows h*128:(h+1)*128 of the Wo contraction) are summed on
host. The kernel is HBM-bound: ~136 MB/core at ~360 GB/s.

Device layout per core (pair p = local_head * 32 + batch, 64 pairs):
  - K cache arrives TRANSPOSED per pair: [d=128 partitions, kv] so score
    matmuls contract d on partitions; V arrives natural [kv, d] chunked
    so context matmuls contract kv on partitions.
  - scores per pair live as [128 kv-in-chunk, 16 chunks] in PSUM;
    softmax runs without max-subtraction (scores are O(5) here), the
    per-pair normalizer is assembled via a ones-vector matmul over the
    activation accumulator and applied at the very end, batched over
    all 64 pairs.
  - The appended (new-token) K/V column is handled separately: its
    score via a [1,1] matmul per pair into a shared [1,64] PSUM row,
    its V contribution via a broadcast outer-product matmul.
"""

import numpy as np
import ml_dtypes
from contextlib import ExitStack

import concourse.bass as bass
import concourse.bacc as bacc
import concourse.mybir as mybir
import concourse.tile as tile

F32 = mybir.dt.float32
BF16 = mybir.dt.bfloat16
AF = mybir.ActivationFunctionType

# Full-problem dims
B = 32          # batch
S = 1           # new tokens
E = 2048        # embed
H = 16          # heads
D = 128         # head dim
KV = 2048       # cache length
N_CORES = 8
HL = H // N_CORES   # local heads per core = 2
NEG = -1.0e30


def build_mha_body(tc, aps, dims, masked=True):
    """Emit the per-core MHA program into TileContext tc.

    aps: dict of bass.AP keyed by tensor name (see build_nc).
    dims: (b, hl, kv, e, eo)
    """
    nc = tc.nc
    b, hl, kv, e, eo = dims
    np_ = hl * b            # pairs
    nkc = kv // 128         # kv chunks
    nec = e // 128          # contraction chunks for projections

    xtb, wq, wqkv, bqkv, wo, ktc, vc = (
        aps["xtb"], aps["wq"], aps["wqkv"], aps["bqkv"],
        aps["wo"], aps["ktc"], aps["vc"])
    lmask = aps.get("lmask")
    outp, ktnew, vtnew = aps["outp"], aps["ktnew"], aps["vtnew"]

    with ExitStack() as ctx:
        singles = ctx.enter_context(tc.tile_pool(name="singles", bufs=1))
        wpool = ctx.enter_context(tc.tile_pool(name="wpool", bufs=1))
        kvpool = ctx.enter_context(tc.tile_pool(name="kvpool", bufs=18))
        epool = ctx.enter_context(tc.tile_pool(name="epool", bufs=12))
        psacc = ctx.enter_context(tc.tile_pool(name="psacc", bufs=1, space="PSUM"))

        # ---- constants / weights ----
        wq_sb = []
        for l in range(hl):
            t = wpool.tile([128, nec, D], BF16, tag=f"wq{l}")
            nc.sync.dma_start(out=t, in_=wq[l].rearrange("e (c d) -> e c d", d=D))
            wq_sb.append(t)
        xtb_sb = singles.tile([128, nec, b], BF16, tag="xtb")
        nc.sync.dma_start(out=xtb_sb, in_=xtb)
        w_sb = []
        for i in range(2 * hl):
            wt = wpool.tile([128, nec, D], BF16, tag=f"w{i}")
            w_sb.append(wt)
        wo_sb = wpool.tile([128, hl, eo], BF16, tag="wo")
        b_sb = singles.tile([D, 3 * hl], F32, tag="b")
        nc.sync.dma_start(out=b_sb, in_=bqkv[:, :])
        if masked:
            lm_sb = singles.tile([128, b, nkc], F32, tag="lm")
            nc.sync.dma_start(out=lm_sb, in_=lmask)
        ones_c = singles.tile([128, 1], F32, tag="ones_c")
        nc.vector.memset(ones_c, 1.0)
        ones_r = singles.tile([1, 128], F32, tag="ones_r")
        nc.vector.memset(ones_r, 1.0)

        # ---- QKV projection: qt/kt/vt [d=128, pairs] (pair = l*b + n) ----
        kt = singles.tile([D, np_], F32, tag="kt")
        vt = singles.tile([D, np_], F32, tag="vt")
        qtb = singles.tile([D, np_], BF16, tag="qtb")
        ktb = singles.tile([D, np_], BF16, tag="ktb")
        def proj_one(pool, t_, l, dest):
            # t_ = 1 (k) or 2 (v); bf16 weights, fp32 PSUM accumulation
            i = (t_ - 1) * hl + l
            ps = pool.tile([D, b], F32, tag="qkv")
            for c in range(nec):
                nc.tensor.matmul(
                    ps, w_sb[i][:, c, :], xtb_sb[:, c, :],
                    start=(c == 0), stop=(c == nec - 1))
            nc.vector.tensor_scalar_add(
                dest[:, l * b:(l + 1) * b], ps, b_sb[:, t_ * hl + l:t_ * hl + l + 1])

        # only the q projection gates the attention loop, and q is consumed
        # exclusively as bf16 by the score matmuls, so it runs fully in bf16;
        # k/v projections and the snew matmuls slot into the loop's idle time
        with tc.tile_pool(name="psproj", bufs=2, space="PSUM") as psproj:
            for l in range(hl):
                ps = psproj.tile([D, b], F32, tag="qkv")
                for c in range(nec):
                    nc.tensor.matmul(
                        ps, wq_sb[l][:, c, :], xtb_sb[:, c, :],
                        start=(c == 0), stop=(c == nec - 1))
                nc.vector.tensor_scalar_add(
                    qtb[:, l * b:(l + 1) * b], ps, b_sb[:, l:l + 1])

        # ---- attention over the cache, pair by pair ----
        ctxp = psacc.tile([D, np_], F32, tag="ctx")     # unnormalized context
        snew = psacc.tile([1, np_], F32, tag="snew")    # new-token scores
        accall = singles.tile([128, np_], F32, tag="accall")  # partial exp sums

        with tc.tile_pool(name="pspair", bufs=5, space="PSUM") as pspair:
            for p in range(np_):
                bi = p % b
                ktile = kvpool.tile([128, kv], BF16, tag="ktile")
                nc.sync.dma_start(out=ktile, in_=ktc[p])
                vtile = kvpool.tile([128, kv], BF16, tag="vtile")
                nc.sync.dma_start(out=vtile, in_=vc[p])

                sp = pspair.tile([128, nkc], F32, tag="sp")
                for c in range(nkc):
                    nc.tensor.matmul(
                        sp[:, c:c + 1], ktile[:, c * 128:(c + 1) * 128],
                        qtb[:, p:p + 1], start=True, stop=True)

                ep = epool.tile([128, nkc], BF16, tag="ep")
                if masked:
                    sm = epool.tile([128, nkc], F32, tag="sm")
                    nc.vector.tensor_add(sm, sp, lm_sb[:, bi, :])
                    nc.scalar.activation(
                        ep, sm, AF.Exp, accum_out=accall[:, p:p + 1])
                else:
                    nc.scalar.activation(
                        ep, sp, AF.Exp, accum_out=accall[:, p:p + 1])

                for c in range(nkc):
                    nc.tensor.matmul(
                        ctxp[:, p:p + 1], vtile[:, c * 128:(c + 1) * 128],
                        ep[:, c:c + 1], start=(c == 0), stop=(c == nkc - 1))

                # k/v projections are off the critical path: their weight
                # DMAs and PE work are spread through the loop in small bites
                if p == 2:
                    nc.sync.dma_start(
                        out=w_sb[0],
                        in_=wqkv[0].rearrange("e (c d) -> e c d", d=D))
                    nc.sync.dma_start(
                        out=w_sb[1],
                        in_=wqkv[1].rearrange("e (c d) -> e c d", d=D))
                elif p == 5:
                    nc.sync.dma_start(
                        out=w_sb[2],
                        in_=wqkv[2].rearrange("e (c d) -> e c d", d=D))
                    nc.sync.dma_start(
                        out=w_sb[3],
                        in_=wqkv[3].rearrange("e (c d) -> e c d", d=D))
                elif p == 8:
                    es_proj = ExitStack()
                    psproj2 = es_proj.enter_context(
                        tc.tile_pool(name="psproj2", bufs=1, space="PSUM"))
                    proj_one(psproj2, 1, 0, kt)
                elif p == 12:
                    proj_one(psproj2, 1, 1, kt)
                    nc.vector.tensor_copy(ktb, kt)
                elif p == 16:
                    proj_one(psproj2, 2, 0, vt)
                elif p == 20:
                    proj_one(psproj2, 2, 1, vt)
                    es_proj.close()
                elif p == 24:
                    # snew[p] = k_new . q: elementwise product then a single
                    # ones-matmul partition reduction
                    qk = singles.tile([D, np_], F32, tag="qk")
                    nc.vector.tensor_mul(qk, ktb, qtb)
                    nc.tensor.matmul(snew, ones_c, qk, start=True, stop=True)
                elif p == 28:
                    nc.sync.dma_start(
                        out=wo_sb, in_=wo.rearrange("l d e -> d l e"))

        # ---- batched softmax tail ----
        ctxn = singles.tile([128, np_], BF16, tag="ctxn")
        with tc.tile_pool(name="pstail", bufs=1, space="PSUM") as pstail:
            sums_ps = pstail.tile([1, np_], F32, tag="sums")
            nc.tensor.matmul(sums_ps, ones_c, accall, start=True, stop=True)
            enew = singles.tile([1, np_], F32, tag="enew")
            nc.scalar.activation(enew, snew, AF.Exp)
            tot = singles.tile([1, np_], F32, tag="tot")
            nc.vector.tensor_add(tot, sums_ps, enew)
            rec = singles.tile([1, np_], F32, tag="rec")
            nc.vector.reciprocal(rec, tot)
            rbc = pstail.tile([128, np_], F32, tag="rbc")
            nc.tensor.matmul(rbc, ones_r, rec, start=True, stop=True)
            ebc = pstail.tile([128, np_], F32, tag="ebc")
            nc.tensor.matmul(ebc, ones_r, enew, start=True, stop=True)
            ctxe = singles.tile([128, np_], F32, tag="ctxe")
            nc.vector.tensor_mul(ctxe, vt, ebc)
            ctxs = singles.tile([128, np_], F32, tag="ctxs")
            nc.vector.tensor_add(ctxs, ctxp, ctxe)
            nc.vector.tensor_mul(ctxn, ctxs, rbc)   # bf16 for the out proj

        # ---- output projection (partial: this core's Wo rows) ----
        with tc.tile_pool(name="psout", bufs=1, space="PSUM") as psout:
            op_ps = psout.tile([b, eo], F32, tag="op")
            for l in range(hl):
                for n0 in range(0, eo, 512):
                    w512 = min(512, eo - n0)
                    nc.tensor.matmul(
                        op_ps[:, n0:n0 + w512], ctxn[:, l * b:(l + 1) * b],
                        wo_sb[:, l, n0:n0 + w512],
                        start=(l == 0), stop=(l == hl - 1))
            out_sb = singles.tile([b, eo], F32, tag="out")
            nc.vector.tensor_copy(out_sb, op_ps)
            nc.sync.dma_start(out=outp, in_=out_sb)
        nc.sync.dma_start(out=ktnew, in_=kt)
        nc.sync.dma_start(out=vtnew, in_=vt)


def build_nc(b=B, hl=HL, kv=KV, e=E, eo=E, masked=True):
    np_ = hl * b
    nkc = kv // 128
    nc = bacc.Bacc(None)
    t = {}
    t["xtb"] = nc.dram_tensor("xtb", [128, e // 128, b], BF16, kind="ExternalInput")
    t["wq"] = nc.dram_tensor("wq", [hl, 128, (e // 128) * D], BF16, kind="ExternalInput")
    t["wqkv"] = nc.dram_tensor("wqkv", [2 * hl, 128, (e // 128) * D], BF16, kind="ExternalInput")
    t["bqkv"] = nc.dram_tensor("bqkv", [D, 3 * hl], F32, kind="ExternalInput")
    t["wo"] = nc.dram_tensor("wo", [hl, D, eo], BF16, kind="ExternalInput")
    if masked:
        t["lmask"] = nc.dram_tensor("lmask", [128, b, nkc], F32, kind="ExternalInput")
    t["ktc"] = nc.dram_tensor("ktc", [np_, D, kv], BF16, kind="ExternalInput")
    t["vc"] = nc.dram_tensor("vc", [np_, 128, kv], BF16, kind="ExternalInput")
    t["outp"] = nc.dram_tensor("outp", [b, eo], F32, kind="ExternalOutput")
    t["ktnew"] = nc.dram_tensor("ktnew", [D, np_], F32, kind="ExternalOutput")
    t["vtnew"] = nc.dram_tensor("vtnew", [D, np_], F32, kind="ExternalOutput")
    with tile.TileContext(nc) as tc:
        build_mha_body(tc, {k: v[:] for k, v in t.items()}, (b, hl, kv, e, eo),
                       masked=masked)
    nc.finalize()
    return nc


def make_core_inputs(core, X, kcache, vcache, mask, Wqkv, bqkv):
    """Host-side shard prep for one core (numpy, full-problem dims)."""
    inv_sqrt_d = np.float32(1.0 / np.sqrt(D))
    h0 = core * HL
    heads = list(range(h0, h0 + HL))

    w_slices = []
    b_slices = []
    for t in range(3):          # q, k, v
        for h in heads:
            ws = np.ascontiguousarray(Wqkv[:, t * E + h * D: t * E + (h + 1) * D])
            bs = np.ascontiguousarray(bqkv[t * E + h * D: t * E + (h + 1) * D])
            if t == 0:
                ws = ws * inv_sqrt_d
                bs = bs * inv_sqrt_d
            w_slices.append(ws)
            b_slices.append(bs)
    # chunk-transposed so each SBUF partition row is one contiguous DMA run:
    # [E, D] -> [e_in_chunk=128, chunk, D]
    packed = [w.reshape(E // 128, 128, D).transpose(1, 0, 2).reshape(128, -1)
              for w in w_slices]
    wq_h = np.stack(packed[:HL], axis=0).astype(ml_dtypes.bfloat16)
    wqkv_h = np.stack(packed[HL:], axis=0).astype(ml_dtypes.bfloat16)
    bqkv_h = np.stack(b_slices, axis=1)                       # [D, 3*HL]

    lmask_h = np.ascontiguousarray(
        np.where(mask == 0, np.float32(NEG), np.float32(0.0)).astype(np.float32)
        .reshape(B, KV // 128, 128).transpose(2, 0, 1))     # [128, B, NKC]

    bf = ml_dtypes.bfloat16
    ktc_h = np.ascontiguousarray(
        kcache[:, heads].astype(bf).transpose(1, 0, 3, 2)).reshape(HL * B, D, KV)
    # V: [kv, d] -> [k_in_chunk=128, chunk*d] so DMA runs are contiguous and
    # SBUF cols [c*128:(c+1)*128] give the [kv=128, d=128] ctx-matmul tile
    vc_h = np.ascontiguousarray(
        vcache[:, heads].astype(bf).reshape(B, HL, KV // 128, 128, D)
        .transpose(1, 0, 3, 2, 4)).reshape(HL * B, 128, KV)
    xt_h = np.ascontiguousarray(
        X.reshape(B, E).T.reshape(E // 128, 128, B).transpose(1, 0, 2))
    return {
        "xtb": xt_h.astype(ml_dtypes.bfloat16),
        "wq": wq_h,
        "wqkv": wqkv_h,
        "bqkv": bqkv_h,
        "wo": None,   # filled by caller (needs Wo)
        "lmask": lmask_h,
        "ktc": ktc_h,
        "vc": vc_h,
    }


_NC_CACHE = {}


def _get_nc(masked):
    key = "m" if masked else "f"
    if key not in _NC_CACHE:
        _NC_CACHE[key] = build_nc(masked=masked)
    return _NC_CACHE[key]


def kernel(X, kcache, vcache, mask, Wqkv, bqkv, Wo, bo, _trace=False):
    from concourse.bass_utils import run_bass_kernel_spmd

    X = np.asarray(X, dtype=np.float32)
    kcache = np.asarray(kcache, dtype=np.float32)
    vcache = np.asarray(vcache, dtype=np.float32)
    mask = np.asarray(mask, dtype=np.float32)
    Wqkv = np.asarray(Wqkv, dtype=np.float32)
    bqkv = np.asarray(bqkv, dtype=np.float32)
    Wo = np.asarray(Wo, dtype=np.float32)
    bo = np.asarray(bo, dtype=np.float32)

    masked = True   # PSUM-direct exp (masked=False) measured slower
    nc = _get_nc(masked)
    in_maps = []
    for core in range(N_CORES):
        m = make_core_inputs(core, X, kcache, vcache, mask, Wqkv, bqkv)
        if not masked:
            m.pop("lmask")
        h0 = core * HL
        m["wo"] = np.ascontiguousarray(
            Wo[h0 * D:(h0 + HL) * D].reshape(HL, D, E)).astype(ml_dtypes.bfloat16)
        in_maps.append(m)

    res = run_bass_kernel_spmd(nc, in_maps, list(range(N_CORES)), trace=_trace)
    outs = res.results

    out = np.zeros((B, E), dtype=np.float32)
    K_new = np.empty((B, H, S, D), dtype=np.float32)
    V_new = np.empty((B, H, S, D), dtype=np.float32)
    for core in range(N_CORES):
        r = outs[core]
        out += r["outp"]
        for l in range(HL):
            K_new[:, core * HL + l, 0, :] = r["ktnew"][:, l * B:(l + 1) * B].T
            V_new[:, core * HL + l, 0, :] = r["vtnew"][:, l * B:(l + 1) * B].T
    out = (out + bo).reshape(B, S, E)

    Kc = np.concatenate([kcache, K_new], axis=2)
    Vc = np.concatenate([vcache, V_new], axis=2)
    mask_new = np.concatenate(
        [mask, np.ones((B, S), dtype=mask.dtype)], axis=-1)
    if _trace:
        return (out, Kc, Vc, mask_new), res
    return out, Kc, Vc, mask_new
